# revision 42
# baseline (speedup 1.0000x reference)
"""Multi-head attention (B=4, S=2048, D=1024, H=16) on 8 NeuronCores.

Sharding: core c -> (batch b = c//2, head-group g = c%2 of 8 heads).
Per-core: column-parallel fused qkv projection for its 8 heads,
flash-style attention (scores kept transposed: k on partitions so
softmax denominators come from a fused ones-column in the PV matmul),
row-parallel out-projection. The two partial outputs per batch are
summed on the host along with b_out.

v2: all matmul operands bf16 (host-cast, halves DMA), mask shipped as
bf16 0/1 so the DVE tensor_tensor runs in its 2-byte fast path, ACT
does exp only during attention (spills moved to Pool, phase-A psum
copies to DVE), m01 DMA issued during phase A.
"""
import sys

if "/opt/trn_rl_repo" not in sys.path:
    sys.path.insert(0, "/opt/trn_rl_repo")

import numpy as np

B, S, D, H = 4, 2048, 1024, 16
DH = D // H          # 64
HPC = H // 2         # 8 heads per core
CD = HPC * DH        # 512 local head-dims per core
NCORES = 8

_CACHE = {}


def _split_multiwait(nc):
    """walrus in this container accepts ONE sync wait per instruction;
    hoist extras onto injected same-engine EventSemaphore carriers."""
    import concourse.mybir as mybir

    for fn in nc.m.functions:
        for bb in fn.blocks:
            if not any(
                i.sync_info is not None and i.sync_info.on_wait
                and len(i.sync_info.on_wait) > 1
                for i in bb.instructions
            ):
                continue
            newlist = []
            for inst in bb.instructions:
                si = inst.sync_info
                if si is not None and si.on_wait and len(si.on_wait) > 1:
                    waits = list(si.on_wait)
                    for w in waits[:-1]:
                        ev = mybir.InstEventSemaphore(
                            name=nc.get_next_instruction_name(), ins=[], outs=[])
                        ev.engine = inst.engine
                        ev.sync_info = mybir.SyncInfo(on_wait=[w], on_update=[])
                        newlist.append(ev)
                    inst.sync_info = mybir.SyncInfo(
                        on_wait=[waits[-1]], on_update=list(si.on_update))
                newlist.append(inst)
            try:
                bb.instructions = newlist
            except Exception:
                bb.instructions.clear()
                bb.instructions.extend(newlist)


def build_nc(s=S):
    import concourse.bass as bass
    import concourse.mybir as mybir
    from concourse.tile import TileContext

    F32 = mybir.dt.float32
    F32R = mybir.dt.float32r
    BF16 = mybir.dt.bfloat16
    EXP = mybir.ActivationFunctionType.Exp
    MULT = mybir.AluOpType.mult

    n_sc = s // 128            # s-chunks of 128
    n_st = s // 512            # s-tiles of 512
    n_kc = s // 128            # k chunks (128 each)
    fd_q = min(512, s)         # q-tile width for attention inner loop
    n_qh = s // fd_q           # q tiles
    VW = CD + HPC              # vones row-chunk width (8 heads x 65)

    nc = bass.Bass("TRN2", num_devices=NCORES)

    xT = nc.declare_dram_parameter("xT", [D, s], BF16, isOutput=False)
    wqk = nc.declare_dram_parameter("wqk", [D, 2 * CD], BF16, isOutput=False)
    wv = nc.declare_dram_parameter("wv", [D, CD], BF16, isOutput=False)
    bqk = nc.declare_dram_parameter("bqk", [1, 2 * CD], BF16, isOutput=False)
    bv = nc.declare_dram_parameter("bv", [1, CD], BF16, isOutput=False)
    m01 = nc.declare_dram_parameter("m01", [s, s], BF16, isOutput=False)
    wout = nc.declare_dram_parameter("wout", [CD, D], BF16, isOutput=False)
    ones = nc.declare_dram_parameter("ones", [1, 512], BF16, isOutput=False)
    y = nc.declare_dram_parameter("y", [s, D], F32, isOutput=True)

    with TileContext(nc) as tc:
        with tc.tile_pool(name="persist", bufs=1) as pp:
            qkT = pp.tile([128, 8 * s], BF16, tag="qkT")       # [1024 c, s]
            vones = pp.tile([128, n_sc * VW], BF16, tag="vones")
            m01t = pp.tile([128, n_kc * s], BF16, tag="m01")

            # ---------------- phase A: qkv projection ----------------
            with tc.tile_pool(name="poolA", bufs=1) as pa, \
                 tc.tile_pool(name="psA", bufs=8, space="PSUM") as psA:
                xt = pa.tile([128, 8 * s], BF16, tag="xt")
                wqkt = pa.tile([128, 8 * 2 * CD], BF16, tag="wqkt")
                wvt = pa.tile([128, 8 * CD], BF16, tag="wvt")
                ones_row = pa.tile([1, 512], BF16, tag="ones")
                bqk_t = pa.tile([1, 2 * CD], BF16, tag="bqk")
                bv_t = pa.tile([1, CD], BF16, tag="bv")

                nc.sync.dma_start(out=ones_row[:], in_=ones[:])
                nc.sync.dma_start(out=bqk_t[:], in_=bqk[:])
                nc.sync.dma_start(out=bv_t[:], in_=bv[:])
                # x + qk weights first (feed the ct loop asap); x chunks
                # alternate queues so neither serializes the whole 4MB
                for dc in range(8):
                    nc.scalar.dma_start(
                        out=wqkt[:, dc * 2 * CD:(dc + 1) * 2 * CD],
                        in_=wqk[dc * 128:(dc + 1) * 128, :])
                    xeng = nc.sync if dc % 2 == 0 else nc.scalar
                    xeng.dma_start(out=xt[:, dc * s:(dc + 1) * s],
                                   in_=xT[dc * 128:(dc + 1) * 128, :])
                for dc in range(8):
                    nc.scalar.dma_start(out=wvt[:, dc * CD:(dc + 1) * CD],
                                        in_=wv[dc * 128:(dc + 1) * 128, :])
                # mask: needed only at attention start; queue behind x,
                # split across both hwdge queues
                for kc in range(n_kc):
                    eng = nc.sync if kc % 2 == 0 else nc.scalar
                    eng.dma_start(out=m01t[:, kc * s:(kc + 1) * s],
                                  in_=m01[kc * 128:(kc + 1) * 128, :])
                # ones columns of vones (the rest is overwritten below)
                vones_cols = vones[:].rearrange(
                    "p (ch e) -> p ch e", e=DH + 1)[:, :, DH:DH + 1]
                nc.gpsimd.memset(vones_cols, 1.0)

                # q/k: qkT[c, :] = (W.T x.T), c-tiles of 128
                for ct in range(8):
                    pst = [psA.tile([128, 512], F32, tag="pa",
                                    name=f"psqk_{ct}_{st}")
                           for st in range(n_st)]
                    for dc in range(8):
                        wsl = wqkt[:, dc * 2 * CD + ct * 128:
                                   dc * 2 * CD + (ct + 1) * 128]
                        for st in range(n_st):
                            nc.tensor.matmul(
                                pst[st][:],
                                lhsT=wsl,
                                rhs=xt[:, dc * s + st * 512:
                                       dc * s + (st + 1) * 512],
                                start=(dc == 0), stop=False)
                    for st in range(n_st):
                        nc.tensor.matmul(
                            pst[st][:],
                            lhsT=bqk_t[0:1, ct * 128:(ct + 1) * 128],
                            rhs=ones_row[0:1, :],
                            start=False, stop=True)
                        nc.scalar.copy(
                            out=qkT[:, ct * s + st * 512:ct * s + (st + 1) * 512],
                            in_=pst[st][:])

                # v: natural [s, c] layout, s-chunks of 128, fused ones col
                for scg in range(n_sc // 4):
                    psv = [psA.tile([128, 512], F32, tag="pa",
                                    name=f"psv_{scg}_{i}")
                           for i in range(4)]
                    for dc in range(8):
                        for sci in range(4):
                            sc = scg * 4 + sci
                            nc.tensor.matmul(
                                psv[sci][:],
                                lhsT=xt[:, dc * s + sc * 128:
                                        dc * s + (sc + 1) * 128],
                                rhs=wvt[:, dc * CD:(dc + 1) * CD],
                                start=(dc == 0), stop=False)
                    for sci in range(4):
                        sc = scg * 4 + sci
                        nc.tensor.matmul(
                            psv[sci][:],
                            lhsT=ones_row[0:1, 0:128],
                            rhs=bv_t[0:1, :],
                            start=False, stop=True)
                        dst = vones[:, sc * VW:(sc + 1) * VW].rearrange(
                            "p (h e) -> p h e", e=DH + 1)[:, :, 0:DH]
                        src = psv[sci][:].rearrange("p (h e) -> p h e", e=DH)
                        nc.vector.tensor_copy(dst, src)

            # ---------------- phase B: attention ----------------
            with tc.tile_pool(name="poolB", bufs=1) as pb:
                ctxT = pb.tile([128, 4 * s], BF16, tag="ctxT")   # [512 c, s]
                woutt = pb.tile([128, 4 * D], BF16, tag="wout")
                for ct in range(4):
                    nc.scalar.dma_start(out=woutt[:, ct * D:(ct + 1) * D],
                                        in_=wout[ct * 128:(ct + 1) * 128, :])
                with (
                    tc.tile_pool(name="poolE", bufs=6) as pe,
                    tc.tile_pool(name="poolBc", bufs=2) as pbc,
                    tc.tile_pool(name="psB_st", bufs=2, space="PSUM") as ps_st,
                    tc.tile_pool(name="psB_ctx", bufs=4, space="PSUM") as ps_ctx,
                ):
                    def emit_norm(hp, rs_p, rcp_p):
                        # normalize pair hp: ctxT[c, q] *= 1/rowsum,
                        # broadcasting the bf16 reciprocal rows over the 64
                        # partitions of each head with a stride-0 DMA
                        with nc.allow_low_precision(
                                reason="recip feeds bf16 prob scale"):
                            nc.vector.reciprocal(rcp_p[:], rs_p[:])
                        rcpb = pbc.tile([2 * n_qh, fd_q], BF16, tag="rcpb",
                                        bufs=4, name=f"rcpb_{hp}")
                        nc.vector.tensor_copy(rcpb[:], rcp_p[:])
                        for qh in range(n_qh):
                            bcp = pbc.tile([128, fd_q], BF16, tag="bcp",
                                           bufs=8, name=f"bcp_{hp}_{qh}")
                            for hi in range(2):
                                r = rcpb[hi * n_qh + qh:hi * n_qh + qh + 1, :]
                                rep = bass.AP(r.tensor, r.offset,
                                              [list(r.ap[0]), [0, 64],
                                               [1, fd_q]])
                                nc.sync.dma_start(
                                    out=bcp[hi * 64:(hi + 1) * 64, :], in_=rep)
                            sl = ctxT[:, hp * s + qh * fd_q:
                                      hp * s + (qh + 1) * fd_q]
                            nc.vector.tensor_tensor(sl, sl, bcp[:], MULT)

                    norm_pending = None
                    for hp in range(4):
                        h0, h1 = 2 * hp, 2 * hp + 1
                        kt_off = (4 + hp) * s   # K pair c-tile offset in qkT
                        qt_off = hp * s         # Q pair c-tile offset
                        rs_p = pbc.tile([2 * n_qh, fd_q], F32, tag="rsp",
                                        bufs=4, name=f"rs_{hp}")
                        rcp_p = pbc.tile([2 * n_qh, fd_q], F32, tag="rcpp",
                                         bufs=4, name=f"rcp_{hp}")
                        for qh in range(n_qh):
                            # previous pair's normalize drops in here, hidden
                            # under this pair's attention
                            if qh == 1 and norm_pending is not None:
                                emit_norm(*norm_pending)
                                norm_pending = None
                            ctx = [ps_ctx.tile([DH + 1, fd_q], F32, tag="ctx",
                                               name=f"ctx_{hp}_{qh}_{hi}")
                                   for hi in range(2)]

                            def emit_pv(kc, e2):
                                first, last = kc == 0, kc == n_kc - 1
                                for hi, h in enumerate((h0, h1)):
                                    nc.tensor.matmul(
                                        ctx[hi][:],
                                        lhsT=vones[:, kc * VW + h * (DH + 1):
                                                   kc * VW +
                                                   (h + 1) * (DH + 1)],
                                        rhs=e2[:, hi * 512:(hi + 1) * 512],
                                        start=first, stop=last)

                            # software pipeline: PV for k-chunk kc-1 issues
                            # after the scores for kc, so the in-order PE
                            # queue never blocks on the exp+mask round trip.
                            # Scores for both heads land in one psum tile
                            # (cols 0:512 head0 / 512:1024 head1) so one exp
                            # and one masked multiply cover the pair.
                            prev = None
                            for kc in range(n_kc):
                                pss = ps_st.tile([128, 2 * fd_q], F32,
                                                 tag="st")
                                for hi in range(2):
                                    r0, r1 = (0, 64) if hi == 0 else (64, 128)
                                    nc.tensor.matmul(
                                        pss[:, hi * 512:(hi + 1) * 512],
                                        lhsT=qkT[r0:r1,
                                                 kt_off + kc * 128:
                                                 kt_off + (kc + 1) * 128],
                                        rhs=qkT[r0:r1,
                                                qt_off + qh * fd_q:
                                                qt_off + (qh + 1) * fd_q],
                                        start=True, stop=True,
                                        tile_position=(r0, 0))
                                e = pe.tile([128, 2 * fd_q], BF16, tag="e")
                                nc.scalar.activation(e[:], pss[:], EXP)
                                mb = m01t[:, kc * s + qh * fd_q:
                                          kc * s + (qh + 1) * fd_q]
                                msl2 = bass.AP(mb.tensor, mb.offset,
                                               [list(mb.ap[0]), [0, 2],
                                                [1, fd_q]])
                                e2v = e[:].rearrange("p (a b) -> p a b", a=2)
                                nc.vector.tensor_tensor(e2v, e2v, msl2, MULT)
                                if prev is not None:
                                    emit_pv(kc - 1, prev)
                                prev = e
                            emit_pv(n_kc - 1, prev)
                            # spill unnormalized ctx + rowsums
                            for hi in range(2):
                                stg = pbc.tile([1, fd_q], F32, tag="stg",
                                               name=f"rstg_{hp}_{qh}_{hi}")
                                nc.vector.tensor_copy(stg[:],
                                                      ctx[hi][DH:DH + 1, :])
                                nc.sync.dma_start(
                                    out=rs_p[hi * n_qh + qh:hi * n_qh + qh + 1, :],
                                    in_=stg[:])
                                nc.vector.tensor_copy(
                                    ctxT[hi * 64:(hi + 1) * 64,
                                         hp * s + qh * fd_q:
                                         hp * s + (qh + 1) * fd_q],
                                    ctx[hi][0:DH, :])

                        norm_pending = (hp, rs_p, rcp_p)
                    emit_norm(*norm_pending)

                # ---------------- phase C: out projection ----------------
                with (
                    tc.tile_pool(name="poolC", bufs=2) as pc,
                    tc.tile_pool(name="psC", bufs=4, space="PSUM") as psC,
                ):
                    for qc in range(n_sc):
                        ot = pc.tile([128, D], F32, tag="ot")
                        for n in range(2):
                            po = psC.tile([128, 512], F32, tag="po")
                            for ct in range(4):
                                nc.tensor.matmul(
                                    po[:],
                                    lhsT=ctxT[:, ct * s + qc * 128:
                                              ct * s + (qc + 1) * 128
                                              ],
                                    rhs=woutt[:, ct * D + n * 512:
                                              ct * D + (n + 1) * 512
                                              ],
                                    start=(ct == 0), stop=(ct == 3))
                            nc.scalar.copy(out=ot[:, n * 512:(n + 1) * 512],
                                           in_=po[:])
                        nc.sync.dma_start(
                            out=y[qc * 128:(qc + 1) * 128, :], in_=ot[:])

    _split_multiwait(nc)
    return nc


def _get_nc(s=S):
    if s not in _CACHE:
        _CACHE[s] = build_nc(s)
    return _CACHE[s]


def make_in_maps(x, W_qkv, b_qkv, W_out, mask, s=S):
    import ml_dtypes

    BF = ml_dtypes.bfloat16
    x = np.asarray(x, dtype=np.float32)
    W_qkv = np.asarray(W_qkv, dtype=np.float32)
    b_qkv = np.asarray(b_qkv, dtype=np.float32)
    W_out = np.asarray(W_out, dtype=np.float32)
    mask = np.asarray(mask)
    scale = 1.0 / np.sqrt(DH)
    m01 = np.ascontiguousarray((mask[0, 0] != 0).T.astype(BF))
    in_maps = []
    for c in range(NCORES):
        b, g = c // 2, c % 2
        wq = W_qkv[:, g * CD:(g + 1) * CD] * scale
        wk = W_qkv[:, D + g * CD:D + (g + 1) * CD]
        in_maps.append({
            "xT": np.ascontiguousarray(x[b].T.astype(BF)),
            "wqk": np.ascontiguousarray(
                np.concatenate([wq, wk], axis=1).astype(BF)),
            "wv": np.ascontiguousarray(
                W_qkv[:, 2 * D + g * CD:2 * D + (g + 1) * CD].astype(BF)),
            "bqk": np.ascontiguousarray(np.concatenate(
                [b_qkv[g * CD:(g + 1) * CD] * scale,
                 b_qkv[D + g * CD:D + (g + 1) * CD]])[None, :].astype(BF)),
            "bv": np.ascontiguousarray(
                b_qkv[2 * D + g * CD:2 * D + (g + 1) * CD][None, :].astype(BF)),
            "m01": m01,
            "wout": np.ascontiguousarray(
                W_out[g * CD:(g + 1) * CD, :].astype(BF)),
            "ones": np.ones((1, 512), dtype=BF),
        })
    return in_maps


def kernel(x, W_qkv, b_qkv, W_out, b_out, mask):
    from concourse.bass_utils import run_bass_kernel_spmd

    nc = _get_nc(S)
    in_maps = make_in_maps(x, W_qkv, b_qkv, W_out, mask, S)
    res = run_bass_kernel_spmd(nc, in_maps, list(range(NCORES)))
    b_out = np.asarray(b_out, dtype=np.float32)
    y = np.empty((B, S, D), dtype=np.float32)
    for b in range(B):
        y[b] = res.results[2 * b]["y"] + res.results[2 * b + 1]["y"] + b_out
    return y


# revision 44
# speedup vs baseline: 1.0279x; 1.0279x over previous
"""Multi-head attention (B=4, S=2048, D=1024, H=16) on 8 NeuronCores.

Sharding: core c -> (batch b = c//2, head-group g = c%2 of 8 heads).
Per-core: column-parallel fused qkv projection for its 8 heads,
flash-style attention (scores kept transposed: k on partitions so
softmax denominators come from a fused ones-column in the PV matmul),
row-parallel out-projection. The two partial outputs per batch are
summed on the host along with b_out.

Perf notes (876us -> 534us on trn2):
- all matmul operands bf16 (host-cast, halves input DMA); mask shipped
  as bf16 0/1 so the DVE mask-multiply hits its 2-byte fast path.
- attention inner loop (fd_q=512): both heads' scores land in one
  [128,1024] psum tile via quadrant-packed 64-contraction matmuls, so
  ONE exp on ACT and ONE rep-AP masked multiply on DVE cover the pair;
  ACT runs ~90% busy (exp is the attention floor).
- PV matmuls are deferred one k-chunk behind the scores so the
  in-order PE queue never blocks on the exp+mask round trip.
- scores psum triple-buffered (6 banks) + 2 ctx accumulators = 8.
- softmax normalization: rowsums from a fused ones-column in PV;
  reciprocal + bf16 row-broadcast via stride-0-partition DMA + DVE
  multiply, all deferred into the NEXT head-pair's attention (deep
  tile bufs to avoid WAR hazards on the deferred reads).
- vones ones-columns via memset (a DMA scatter here cost 50us);
  m01/x DMA split across both hwdge queues; wout prefetched at
  attention start; phase C writes y rows with single [128,1024] DMAs.
Failed experiments: Pool (gpsimd) for any elementwise op (3-4x slower
than DVE); co-resident quadrant pairs that write the same psum region
(hardware fault) or read different rhs streams (no gain - the win of
quadrant packing comes from sharing one rhs fetch stream); fp8 /
Schraudolph-approx exp (error budget); reciprocal_approx_fast (walrus
codegen "ISA wrong length").
"""
import sys

if "/opt/trn_rl_repo" not in sys.path:
    sys.path.insert(0, "/opt/trn_rl_repo")

import numpy as np

B, S, D, H = 4, 2048, 1024, 16
DH = D // H          # 64
HPC = H // 2         # 8 heads per core
CD = HPC * DH        # 512 local head-dims per core
NCORES = 8

_CACHE = {}


def _split_multiwait(nc):
    """walrus in this container accepts ONE sync wait per instruction;
    hoist extras onto injected same-engine EventSemaphore carriers."""
    import concourse.mybir as mybir

    for fn in nc.m.functions:
        for bb in fn.blocks:
            if not any(
                i.sync_info is not None and i.sync_info.on_wait
                and len(i.sync_info.on_wait) > 1
                for i in bb.instructions
            ):
                continue
            newlist = []
            for inst in bb.instructions:
                si = inst.sync_info
                if si is not None and si.on_wait and len(si.on_wait) > 1:
                    waits = list(si.on_wait)
                    for w in waits[:-1]:
                        ev = mybir.InstEventSemaphore(
                            name=nc.get_next_instruction_name(), ins=[], outs=[])
                        ev.engine = inst.engine
                        ev.sync_info = mybir.SyncInfo(on_wait=[w], on_update=[])
                        newlist.append(ev)
                    inst.sync_info = mybir.SyncInfo(
                        on_wait=[waits[-1]], on_update=list(si.on_update))
                newlist.append(inst)
            try:
                bb.instructions = newlist
            except Exception:
                bb.instructions.clear()
                bb.instructions.extend(newlist)


def build_nc(s=S):
    import concourse.bass as bass
    import concourse.mybir as mybir
    from concourse.tile import TileContext

    F32 = mybir.dt.float32
    F32R = mybir.dt.float32r
    BF16 = mybir.dt.bfloat16
    EXP = mybir.ActivationFunctionType.Exp
    MULT = mybir.AluOpType.mult

    n_sc = s // 128            # s-chunks of 128
    n_st = s // 512            # s-tiles of 512
    n_kc = s // 128            # k chunks (128 each)
    fd_q = min(512, s)         # q-tile width for attention inner loop
    n_qh = s // fd_q           # q tiles
    VW = CD + HPC              # vones row-chunk width (8 heads x 65)

    nc = bass.Bass("TRN2", num_devices=NCORES)

    xT = nc.declare_dram_parameter("xT", [D, s], BF16, isOutput=False)
    wqk = nc.declare_dram_parameter("wqk", [D, 2 * CD], BF16, isOutput=False)
    wv = nc.declare_dram_parameter("wv", [D, CD], BF16, isOutput=False)
    bqk = nc.declare_dram_parameter("bqk", [1, 2 * CD], BF16, isOutput=False)
    bv = nc.declare_dram_parameter("bv", [1, CD], BF16, isOutput=False)
    m01 = nc.declare_dram_parameter("m01", [s, s], BF16, isOutput=False)
    wout = nc.declare_dram_parameter("wout", [CD, D], BF16, isOutput=False)
    ones = nc.declare_dram_parameter("ones", [1, 512], BF16, isOutput=False)
    y = nc.declare_dram_parameter("y", [s, D], F32, isOutput=True)

    with TileContext(nc) as tc:
        with tc.tile_pool(name="persist", bufs=1) as pp:
            qkT = pp.tile([128, 8 * s], BF16, tag="qkT")       # [1024 c, s]
            vones = pp.tile([128, n_sc * VW], BF16, tag="vones")
            m01t = pp.tile([128, n_kc * s], BF16, tag="m01")

            # ---------------- phase A: qkv projection ----------------
            with tc.tile_pool(name="poolA", bufs=1) as pa, \
                 tc.tile_pool(name="psA", bufs=8, space="PSUM") as psA:
                xt = pa.tile([128, 8 * s], BF16, tag="xt")
                wqkt = pa.tile([128, 8 * 2 * CD], BF16, tag="wqkt")
                wvt = pa.tile([128, 8 * CD], BF16, tag="wvt")
                ones_row = pa.tile([1, 512], BF16, tag="ones")
                bqk_t = pa.tile([1, 2 * CD], BF16, tag="bqk")
                bv_t = pa.tile([1, CD], BF16, tag="bv")

                nc.sync.dma_start(out=ones_row[:], in_=ones[:])
                nc.sync.dma_start(out=bqk_t[:], in_=bqk[:])
                nc.sync.dma_start(out=bv_t[:], in_=bv[:])
                # x + qk weights first (feed the ct loop asap); x chunks
                # alternate queues so neither serializes the whole 4MB
                for dc in range(8):
                    nc.scalar.dma_start(
                        out=wqkt[:, dc * 2 * CD:(dc + 1) * 2 * CD],
                        in_=wqk[dc * 128:(dc + 1) * 128, :])
                    xeng = nc.sync if dc % 2 == 0 else nc.scalar
                    xeng.dma_start(out=xt[:, dc * s:(dc + 1) * s],
                                   in_=xT[dc * 128:(dc + 1) * 128, :])
                for dc in range(8):
                    nc.scalar.dma_start(out=wvt[:, dc * CD:(dc + 1) * CD],
                                        in_=wv[dc * 128:(dc + 1) * 128, :])
                # mask: needed only at attention start; queue behind x,
                # split across both hwdge queues
                for kc in range(n_kc):
                    eng = nc.sync if kc % 2 == 0 else nc.scalar
                    eng.dma_start(out=m01t[:, kc * s:(kc + 1) * s],
                                  in_=m01[kc * 128:(kc + 1) * 128, :])
                # ones columns of vones (the rest is overwritten below)
                vones_cols = vones[:].rearrange(
                    "p (ch e) -> p ch e", e=DH + 1)[:, :, DH:DH + 1]
                nc.gpsimd.memset(vones_cols, 1.0)

                # q/k: qkT[c, :] = (W.T x.T), c-tiles of 128
                for ct in range(8):
                    pst = [psA.tile([128, 512], F32, tag="pa",
                                    name=f"psqk_{ct}_{st}")
                           for st in range(n_st)]
                    for dc in range(8):
                        wsl = wqkt[:, dc * 2 * CD + ct * 128:
                                   dc * 2 * CD + (ct + 1) * 128]
                        for st in range(n_st):
                            nc.tensor.matmul(
                                pst[st][:],
                                lhsT=wsl,
                                rhs=xt[:, dc * s + st * 512:
                                       dc * s + (st + 1) * 512],
                                start=(dc == 0), stop=False)
                    for st in range(n_st):
                        nc.tensor.matmul(
                            pst[st][:],
                            lhsT=bqk_t[0:1, ct * 128:(ct + 1) * 128],
                            rhs=ones_row[0:1, :],
                            start=False, stop=True)
                        nc.scalar.copy(
                            out=qkT[:, ct * s + st * 512:ct * s + (st + 1) * 512],
                            in_=pst[st][:])

                # v: natural [s, c] layout, s-chunks of 128, fused ones col
                for scg in range(n_sc // 4):
                    psv = [psA.tile([128, 512], F32, tag="pa",
                                    name=f"psv_{scg}_{i}")
                           for i in range(4)]
                    for dc in range(8):
                        for sci in range(4):
                            sc = scg * 4 + sci
                            nc.tensor.matmul(
                                psv[sci][:],
                                lhsT=xt[:, dc * s + sc * 128:
                                        dc * s + (sc + 1) * 128],
                                rhs=wvt[:, dc * CD:(dc + 1) * CD],
                                start=(dc == 0), stop=False)
                    for sci in range(4):
                        sc = scg * 4 + sci
                        nc.tensor.matmul(
                            psv[sci][:],
                            lhsT=ones_row[0:1, 0:128],
                            rhs=bv_t[0:1, :],
                            start=False, stop=True)
                        dst = vones[:, sc * VW:(sc + 1) * VW].rearrange(
                            "p (h e) -> p h e", e=DH + 1)[:, :, 0:DH]
                        src = psv[sci][:].rearrange("p (h e) -> p h e", e=DH)
                        nc.vector.tensor_copy(dst, src)

            # ---------------- phase B: attention ----------------
            with tc.tile_pool(name="poolB", bufs=1) as pb:
                ctxT = pb.tile([128, 4 * s], BF16, tag="ctxT")   # [512 c, s]
                woutt = pb.tile([128, 4 * D], BF16, tag="wout")
                for ct in range(4):
                    nc.scalar.dma_start(out=woutt[:, ct * D:(ct + 1) * D],
                                        in_=wout[ct * 128:(ct + 1) * 128, :])
                with (
                    tc.tile_pool(name="poolE", bufs=6) as pe,
                    tc.tile_pool(name="poolBc", bufs=2) as pbc,
                    tc.tile_pool(name="psB_st", bufs=3, space="PSUM") as ps_st,
                    tc.tile_pool(name="psB_ctx", bufs=2, space="PSUM") as ps_ctx,
                ):
                    def emit_norm(hp, rs_p, rcp_p):
                        # normalize pair hp: ctxT[c, q] *= 1/rowsum,
                        # broadcasting the bf16 reciprocal rows over the 64
                        # partitions of each head with a stride-0 DMA
                        with nc.allow_low_precision(
                                reason="recip feeds bf16 prob scale"):
                            nc.vector.reciprocal(rcp_p[:], rs_p[:])
                        rcpb = pbc.tile([2 * n_qh, fd_q], BF16, tag="rcpb",
                                        bufs=4, name=f"rcpb_{hp}")
                        nc.vector.tensor_copy(rcpb[:], rcp_p[:])
                        for qh in range(n_qh):
                            bcp = pbc.tile([128, fd_q], BF16, tag="bcp",
                                           bufs=8, name=f"bcp_{hp}_{qh}")
                            for hi in range(2):
                                r = rcpb[hi * n_qh + qh:hi * n_qh + qh + 1, :]
                                rep = bass.AP(r.tensor, r.offset,
                                              [list(r.ap[0]), [0, 64],
                                               [1, fd_q]])
                                nc.sync.dma_start(
                                    out=bcp[hi * 64:(hi + 1) * 64, :], in_=rep)
                            sl = ctxT[:, hp * s + qh * fd_q:
                                      hp * s + (qh + 1) * fd_q]
                            nc.vector.tensor_tensor(sl, sl, bcp[:], MULT)

                    norm_pending = None
                    for hp in range(4):
                        h0, h1 = 2 * hp, 2 * hp + 1
                        kt_off = (4 + hp) * s   # K pair c-tile offset in qkT
                        qt_off = hp * s         # Q pair c-tile offset
                        rs_p = pbc.tile([2 * n_qh, fd_q], F32, tag="rsp",
                                        bufs=4, name=f"rs_{hp}")
                        rcp_p = pbc.tile([2 * n_qh, fd_q], F32, tag="rcpp",
                                         bufs=4, name=f"rcp_{hp}")
                        for qh in range(n_qh):
                            # previous pair's normalize drops in here, hidden
                            # under this pair's attention
                            if qh == 1 and norm_pending is not None:
                                emit_norm(*norm_pending)
                                norm_pending = None
                            ctx = [ps_ctx.tile([DH + 1, fd_q], F32, tag="ctx",
                                               name=f"ctx_{hp}_{qh}_{hi}")
                                   for hi in range(2)]

                            def emit_pv(kc, e2):
                                first, last = kc == 0, kc == n_kc - 1
                                for hi, h in enumerate((h0, h1)):
                                    nc.tensor.matmul(
                                        ctx[hi][:],
                                        lhsT=vones[:, kc * VW + h * (DH + 1):
                                                   kc * VW +
                                                   (h + 1) * (DH + 1)],
                                        rhs=e2[:, hi * 512:(hi + 1) * 512],
                                        start=first, stop=last)

                            # software pipeline: PV for k-chunk kc-1 issues
                            # after the scores for kc, so the in-order PE
                            # queue never blocks on the exp+mask round trip.
                            # Scores for both heads land in one psum tile
                            # (cols 0:512 head0 / 512:1024 head1) so one exp
                            # and one masked multiply cover the pair.
                            prev = None
                            for kc in range(n_kc):
                                pss = ps_st.tile([128, 2 * fd_q], F32,
                                                 tag="st")
                                for hi in range(2):
                                    r0, r1 = (0, 64) if hi == 0 else (64, 128)
                                    nc.tensor.matmul(
                                        pss[:, hi * 512:(hi + 1) * 512],
                                        lhsT=qkT[r0:r1,
                                                 kt_off + kc * 128:
                                                 kt_off + (kc + 1) * 128],
                                        rhs=qkT[r0:r1,
                                                qt_off + qh * fd_q:
                                                qt_off + (qh + 1) * fd_q],
                                        start=True, stop=True,
                                        tile_position=(r0, 0))
                                e = pe.tile([128, 2 * fd_q], BF16, tag="e")
                                nc.scalar.activation(e[:], pss[:], EXP)
                                mb = m01t[:, kc * s + qh * fd_q:
                                          kc * s + (qh + 1) * fd_q]
                                msl2 = bass.AP(mb.tensor, mb.offset,
                                               [list(mb.ap[0]), [0, 2],
                                                [1, fd_q]])
                                e2v = e[:].rearrange("p (a b) -> p a b", a=2)
                                nc.vector.tensor_tensor(e2v, e2v, msl2, MULT)
                                if prev is not None:
                                    emit_pv(kc - 1, prev)
                                prev = e
                            emit_pv(n_kc - 1, prev)
                            # spill unnormalized ctx + rowsums
                            for hi in range(2):
                                stg = pbc.tile([1, fd_q], F32, tag="stg",
                                               name=f"rstg_{hp}_{qh}_{hi}")
                                nc.vector.tensor_copy(stg[:],
                                                      ctx[hi][DH:DH + 1, :])
                                nc.sync.dma_start(
                                    out=rs_p[hi * n_qh + qh:hi * n_qh + qh + 1, :],
                                    in_=stg[:])
                                nc.vector.tensor_copy(
                                    ctxT[hi * 64:(hi + 1) * 64,
                                         hp * s + qh * fd_q:
                                         hp * s + (qh + 1) * fd_q],
                                    ctx[hi][0:DH, :])

                        norm_pending = (hp, rs_p, rcp_p)
                    emit_norm(*norm_pending)

                # ---------------- phase C: out projection ----------------
                with (
                    tc.tile_pool(name="poolC", bufs=2) as pc,
                    tc.tile_pool(name="psC", bufs=4, space="PSUM") as psC,
                ):
                    for qc in range(n_sc):
                        ot = pc.tile([128, D], F32, tag="ot")
                        for n in range(2):
                            po = psC.tile([128, 512], F32, tag="po")
                            for ct in range(4):
                                nc.tensor.matmul(
                                    po[:],
                                    lhsT=ctxT[:, ct * s + qc * 128:
                                              ct * s + (qc + 1) * 128
                                              ],
                                    rhs=woutt[:, ct * D + n * 512:
                                              ct * D + (n + 1) * 512
                                              ],
                                    start=(ct == 0), stop=(ct == 3))
                            nc.scalar.copy(out=ot[:, n * 512:(n + 1) * 512],
                                           in_=po[:])
                        nc.sync.dma_start(
                            out=y[qc * 128:(qc + 1) * 128, :], in_=ot[:])

    _split_multiwait(nc)
    return nc


def _get_nc(s=S):
    if s not in _CACHE:
        _CACHE[s] = build_nc(s)
    return _CACHE[s]


def make_in_maps(x, W_qkv, b_qkv, W_out, mask, s=S):
    import ml_dtypes

    BF = ml_dtypes.bfloat16
    x = np.asarray(x, dtype=np.float32)
    W_qkv = np.asarray(W_qkv, dtype=np.float32)
    b_qkv = np.asarray(b_qkv, dtype=np.float32)
    W_out = np.asarray(W_out, dtype=np.float32)
    mask = np.asarray(mask)
    scale = 1.0 / np.sqrt(DH)
    m01 = np.ascontiguousarray((mask[0, 0] != 0).T.astype(BF))
    in_maps = []
    for c in range(NCORES):
        b, g = c // 2, c % 2
        wq = W_qkv[:, g * CD:(g + 1) * CD] * scale
        wk = W_qkv[:, D + g * CD:D + (g + 1) * CD]
        in_maps.append({
            "xT": np.ascontiguousarray(x[b].T.astype(BF)),
            "wqk": np.ascontiguousarray(
                np.concatenate([wq, wk], axis=1).astype(BF)),
            "wv": np.ascontiguousarray(
                W_qkv[:, 2 * D + g * CD:2 * D + (g + 1) * CD].astype(BF)),
            "bqk": np.ascontiguousarray(np.concatenate(
                [b_qkv[g * CD:(g + 1) * CD] * scale,
                 b_qkv[D + g * CD:D + (g + 1) * CD]])[None, :].astype(BF)),
            "bv": np.ascontiguousarray(
                b_qkv[2 * D + g * CD:2 * D + (g + 1) * CD][None, :].astype(BF)),
            "m01": m01,
            "wout": np.ascontiguousarray(
                W_out[g * CD:(g + 1) * CD, :].astype(BF)),
            "ones": np.ones((1, 512), dtype=BF),
        })
    return in_maps


def kernel(x, W_qkv, b_qkv, W_out, b_out, mask):
    from concourse.bass_utils import run_bass_kernel_spmd

    nc = _get_nc(S)
    in_maps = make_in_maps(x, W_qkv, b_qkv, W_out, mask, S)
    res = run_bass_kernel_spmd(nc, in_maps, list(range(NCORES)))
    b_out = np.asarray(b_out, dtype=np.float32)
    y = np.empty((B, S, D), dtype=np.float32)
    for b in range(B):
        y[b] = res.results[2 * b]["y"] + res.results[2 * b + 1]["y"] + b_out
    return y


# revision 46
# speedup vs baseline: 1.0294x; 1.0014x over previous
"""Multi-head attention (B=4, S=2048, D=1024, H=16) on 8 NeuronCores.

Sharding: core c -> (batch b = c//2, head-group g = c%2 of 8 heads).
Per-core: column-parallel fused qkv projection for its 8 heads,
flash-style attention (scores kept transposed: k on partitions so
softmax denominators come from a fused ones-column in the PV matmul),
row-parallel out-projection. The two partial outputs per batch are
summed on the host along with b_out.

Perf notes (876us -> 534us on trn2):
- all matmul operands bf16 (host-cast, halves input DMA); mask shipped
  as bf16 0/1 so the DVE mask-multiply hits its 2-byte fast path.
- attention inner loop (fd_q=512): both heads' scores land in one
  [128,1024] psum tile via quadrant-packed 64-contraction matmuls, so
  ONE exp on ACT and ONE rep-AP masked multiply on DVE cover the pair;
  ACT runs ~90% busy (exp is the attention floor).
- PV matmuls are deferred one k-chunk behind the scores so the
  in-order PE queue never blocks on the exp+mask round trip.
- scores psum triple-buffered (6 banks) + 2 ctx accumulators = 8.
- softmax normalization: rowsums from a fused ones-column in PV;
  reciprocal + bf16 row-broadcast via stride-0-partition DMA + DVE
  multiply, all deferred into the NEXT head-pair's attention (deep
  tile bufs to avoid WAR hazards on the deferred reads).
- vones ones-columns via memset (a DMA scatter here cost 50us);
  m01/x DMA split across both hwdge queues; wout prefetched at
  attention start; phase C writes y rows with single [128,1024] DMAs.
Failed experiments: Pool (gpsimd) for any elementwise op (3-4x slower
than DVE); co-resident quadrant pairs that write the same psum region
(hardware fault) or read different rhs streams (no gain - the win of
quadrant packing comes from sharing one rhs fetch stream); fp8 /
Schraudolph-approx exp (error budget); reciprocal_approx_fast (walrus
codegen "ISA wrong length").
"""
import sys

if "/opt/trn_rl_repo" not in sys.path:
    sys.path.insert(0, "/opt/trn_rl_repo")

import numpy as np

B, S, D, H = 4, 2048, 1024, 16
DH = D // H          # 64
HPC = H // 2         # 8 heads per core
CD = HPC * DH        # 512 local head-dims per core
NCORES = 8

_CACHE = {}


def _split_multiwait(nc):
    """walrus in this container accepts ONE sync wait per instruction;
    hoist extras onto injected same-engine EventSemaphore carriers."""
    import concourse.mybir as mybir

    for fn in nc.m.functions:
        for bb in fn.blocks:
            if not any(
                i.sync_info is not None and i.sync_info.on_wait
                and len(i.sync_info.on_wait) > 1
                for i in bb.instructions
            ):
                continue
            newlist = []
            for inst in bb.instructions:
                si = inst.sync_info
                if si is not None and si.on_wait and len(si.on_wait) > 1:
                    waits = list(si.on_wait)
                    for w in waits[:-1]:
                        ev = mybir.InstEventSemaphore(
                            name=nc.get_next_instruction_name(), ins=[], outs=[])
                        ev.engine = inst.engine
                        ev.sync_info = mybir.SyncInfo(on_wait=[w], on_update=[])
                        newlist.append(ev)
                    inst.sync_info = mybir.SyncInfo(
                        on_wait=[waits[-1]], on_update=list(si.on_update))
                newlist.append(inst)
            try:
                bb.instructions = newlist
            except Exception:
                bb.instructions.clear()
                bb.instructions.extend(newlist)


def build_nc(s=S):
    import concourse.bass as bass
    import concourse.mybir as mybir
    from concourse.tile import TileContext

    F32 = mybir.dt.float32
    F32R = mybir.dt.float32r
    BF16 = mybir.dt.bfloat16
    EXP = mybir.ActivationFunctionType.Exp
    MULT = mybir.AluOpType.mult

    n_sc = s // 128            # s-chunks of 128
    n_st = s // 512            # s-tiles of 512
    n_kc = s // 128            # k chunks (128 each)
    fd_q = min(512, s)         # q-tile width for attention inner loop
    n_qh = s // fd_q           # q tiles
    VW = CD + HPC              # vones row-chunk width (8 heads x 65)

    nc = bass.Bass("TRN2", num_devices=NCORES)

    xT = nc.declare_dram_parameter("xT", [D, s], BF16, isOutput=False)
    wqk = nc.declare_dram_parameter("wqk", [D, 2 * CD], BF16, isOutput=False)
    wv = nc.declare_dram_parameter("wv", [D, CD], BF16, isOutput=False)
    bqk = nc.declare_dram_parameter("bqk", [1, 2 * CD], BF16, isOutput=False)
    bv = nc.declare_dram_parameter("bv", [1, CD], BF16, isOutput=False)
    m01 = nc.declare_dram_parameter("m01", [s, s], BF16, isOutput=False)
    wout = nc.declare_dram_parameter("wout", [CD, D], BF16, isOutput=False)
    ones = nc.declare_dram_parameter("ones", [1, 512], BF16, isOutput=False)
    y = nc.declare_dram_parameter("y", [s, D], F32, isOutput=True)

    with TileContext(nc) as tc:
        with tc.tile_pool(name="persist", bufs=1) as pp:
            qkT = pp.tile([128, 8 * s], BF16, tag="qkT")       # [1024 c, s]
            vones = pp.tile([128, n_sc * VW], BF16, tag="vones")
            m01t = pp.tile([128, n_kc * s], BF16, tag="m01")

            # ---------------- phase A: qkv projection ----------------
            with tc.tile_pool(name="poolA", bufs=1) as pa, \
                 tc.tile_pool(name="psA", bufs=8, space="PSUM") as psA:
                xt = pa.tile([128, 8 * s], BF16, tag="xt")
                wqkt = pa.tile([128, 8 * 2 * CD], BF16, tag="wqkt")
                wvt = pa.tile([128, 8 * CD], BF16, tag="wvt")
                ones_row = pa.tile([1, 512], BF16, tag="ones")
                bqk_t = pa.tile([1, 2 * CD], BF16, tag="bqk")
                bv_t = pa.tile([1, CD], BF16, tag="bv")

                nc.sync.dma_start(out=ones_row[:], in_=ones[:])
                nc.sync.dma_start(out=bqk_t[:], in_=bqk[:])
                nc.sync.dma_start(out=bv_t[:], in_=bv[:])
                # x + qk weights first (feed the ct loop asap); x chunks
                # alternate queues so neither serializes the whole 4MB
                for dc in range(8):
                    nc.scalar.dma_start(
                        out=wqkt[:, dc * 2 * CD:(dc + 1) * 2 * CD],
                        in_=wqk[dc * 128:(dc + 1) * 128, :])
                    xeng = nc.sync if dc % 2 == 0 else nc.scalar
                    xeng.dma_start(out=xt[:, dc * s:(dc + 1) * s],
                                   in_=xT[dc * 128:(dc + 1) * 128, :])
                for dc in range(8):
                    nc.scalar.dma_start(out=wvt[:, dc * CD:(dc + 1) * CD],
                                        in_=wv[dc * 128:(dc + 1) * 128, :])
                # mask: needed only at attention start; queue behind x,
                # split across both hwdge queues
                for kc in range(n_kc):
                    eng = nc.sync if kc % 2 == 0 else nc.scalar
                    eng.dma_start(out=m01t[:, kc * s:(kc + 1) * s],
                                  in_=m01[kc * 128:(kc + 1) * 128, :])
                # ones columns of vones (the rest is overwritten below)
                vones_cols = vones[:].rearrange(
                    "p (ch e) -> p ch e", e=DH + 1)[:, :, DH:DH + 1]
                nc.gpsimd.memset(vones_cols, 1.0)

                # q/k: qkT[c, :] = (W.T x.T), c-tiles of 128
                for ct in range(8):
                    pst = [psA.tile([128, 512], F32, tag="pa",
                                    name=f"psqk_{ct}_{st}")
                           for st in range(n_st)]
                    for dc in range(8):
                        wsl = wqkt[:, dc * 2 * CD + ct * 128:
                                   dc * 2 * CD + (ct + 1) * 128]
                        for st in range(n_st):
                            nc.tensor.matmul(
                                pst[st][:],
                                lhsT=wsl,
                                rhs=xt[:, dc * s + st * 512:
                                       dc * s + (st + 1) * 512],
                                start=(dc == 0), stop=False)
                    for st in range(n_st):
                        nc.tensor.matmul(
                            pst[st][:],
                            lhsT=bqk_t[0:1, ct * 128:(ct + 1) * 128],
                            rhs=ones_row[0:1, :],
                            start=False, stop=True)
                        nc.scalar.copy(
                            out=qkT[:, ct * s + st * 512:ct * s + (st + 1) * 512],
                            in_=pst[st][:])

                # v: natural [s, c] layout, s-chunks of 128, fused ones col
                for scg in range(n_sc // 4):
                    psv = [psA.tile([128, 512], F32, tag="pa",
                                    name=f"psv_{scg}_{i}")
                           for i in range(4)]
                    for dc in range(8):
                        for sci in range(4):
                            sc = scg * 4 + sci
                            nc.tensor.matmul(
                                psv[sci][:],
                                lhsT=xt[:, dc * s + sc * 128:
                                        dc * s + (sc + 1) * 128],
                                rhs=wvt[:, dc * CD:(dc + 1) * CD],
                                start=(dc == 0), stop=False)
                    for sci in range(4):
                        sc = scg * 4 + sci
                        nc.tensor.matmul(
                            psv[sci][:],
                            lhsT=ones_row[0:1, 0:128],
                            rhs=bv_t[0:1, :],
                            start=False, stop=True)
                        dst = vones[:, sc * VW:(sc + 1) * VW].rearrange(
                            "p (h e) -> p h e", e=DH + 1)[:, :, 0:DH]
                        src = psv[sci][:].rearrange("p (h e) -> p h e", e=DH)
                        nc.vector.tensor_copy(dst, src)

            # ---------------- phase B: attention ----------------
            with tc.tile_pool(name="poolB", bufs=1) as pb:
                ctxT = pb.tile([128, 4 * s], BF16, tag="ctxT")   # [512 c, s]
                woutt = pb.tile([128, 4 * D], BF16, tag="wout")
                for ct in range(4):
                    nc.scalar.dma_start(out=woutt[:, ct * D:(ct + 1) * D],
                                        in_=wout[ct * 128:(ct + 1) * 128, :])
                with (
                    tc.tile_pool(name="poolE", bufs=8) as pe,
                    tc.tile_pool(name="poolBc", bufs=2) as pbc,
                    tc.tile_pool(name="psB_st", bufs=3, space="PSUM") as ps_st,
                    tc.tile_pool(name="psB_ctx", bufs=2, space="PSUM") as ps_ctx,
                ):
                    def emit_norm(hp, rs_p, rcp_p):
                        # normalize pair hp: ctxT[c, q] *= 1/rowsum,
                        # broadcasting the bf16 reciprocal rows over the 64
                        # partitions of each head with a stride-0 DMA
                        with nc.allow_low_precision(
                                reason="recip feeds bf16 prob scale"):
                            nc.vector.reciprocal(rcp_p[:], rs_p[:])
                        rcpb = pbc.tile([2 * n_qh, fd_q], BF16, tag="rcpb",
                                        bufs=4, name=f"rcpb_{hp}")
                        nc.vector.tensor_copy(rcpb[:], rcp_p[:])
                        for qh in range(n_qh):
                            bcp = pbc.tile([128, fd_q], BF16, tag="bcp",
                                           bufs=8, name=f"bcp_{hp}_{qh}")
                            for hi in range(2):
                                r = rcpb[hi * n_qh + qh:hi * n_qh + qh + 1, :]
                                rep = bass.AP(r.tensor, r.offset,
                                              [list(r.ap[0]), [0, 64],
                                               [1, fd_q]])
                                nc.sync.dma_start(
                                    out=bcp[hi * 64:(hi + 1) * 64, :], in_=rep)
                            sl = ctxT[:, hp * s + qh * fd_q:
                                      hp * s + (qh + 1) * fd_q]
                            nc.vector.tensor_tensor(sl, sl, bcp[:], MULT)

                    norm_pending = None
                    for hp in range(4):
                        h0, h1 = 2 * hp, 2 * hp + 1
                        kt_off = (4 + hp) * s   # K pair c-tile offset in qkT
                        qt_off = hp * s         # Q pair c-tile offset
                        rs_p = pbc.tile([2 * n_qh, fd_q], F32, tag="rsp",
                                        bufs=4, name=f"rs_{hp}")
                        rcp_p = pbc.tile([2 * n_qh, fd_q], F32, tag="rcpp",
                                         bufs=4, name=f"rcp_{hp}")
                        for qh in range(n_qh):
                            # previous pair's normalize drops in here, hidden
                            # under this pair's attention
                            if qh == 1 and norm_pending is not None:
                                emit_norm(*norm_pending)
                                norm_pending = None
                            ctx = [ps_ctx.tile([DH + 1, fd_q], F32, tag="ctx",
                                               name=f"ctx_{hp}_{qh}_{hi}")
                                   for hi in range(2)]

                            def emit_pv(kc, e2):
                                first, last = kc == 0, kc == n_kc - 1
                                for hi, h in enumerate((h0, h1)):
                                    nc.tensor.matmul(
                                        ctx[hi][:],
                                        lhsT=vones[:, kc * VW + h * (DH + 1):
                                                   kc * VW +
                                                   (h + 1) * (DH + 1)],
                                        rhs=e2[:, hi * 512:(hi + 1) * 512],
                                        start=first, stop=last)

                            # software pipeline: PV for k-chunk kc-1 issues
                            # after the scores for kc, so the in-order PE
                            # queue never blocks on the exp+mask round trip.
                            # Scores for both heads land in one psum tile
                            # (cols 0:512 head0 / 512:1024 head1) so one exp
                            # and one masked multiply cover the pair.
                            prev = None
                            for kc in range(n_kc):
                                pss = ps_st.tile([128, 2 * fd_q], F32,
                                                 tag="st")
                                for hi in range(2):
                                    r0, r1 = (0, 64) if hi == 0 else (64, 128)
                                    nc.tensor.matmul(
                                        pss[:, hi * 512:(hi + 1) * 512],
                                        lhsT=qkT[r0:r1,
                                                 kt_off + kc * 128:
                                                 kt_off + (kc + 1) * 128],
                                        rhs=qkT[r0:r1,
                                                qt_off + qh * fd_q:
                                                qt_off + (qh + 1) * fd_q],
                                        start=True, stop=True,
                                        tile_position=(r0, 0))
                                e = pe.tile([128, 2 * fd_q], BF16, tag="e")
                                nc.scalar.activation(e[:], pss[:], EXP)
                                mb = m01t[:, kc * s + qh * fd_q:
                                          kc * s + (qh + 1) * fd_q]
                                msl2 = bass.AP(mb.tensor, mb.offset,
                                               [list(mb.ap[0]), [0, 2],
                                                [1, fd_q]])
                                e2v = e[:].rearrange("p (a b) -> p a b", a=2)
                                nc.vector.tensor_tensor(e2v, e2v, msl2, MULT)
                                if prev is not None:
                                    emit_pv(kc - 1, prev)
                                prev = e
                            emit_pv(n_kc - 1, prev)
                            # spill unnormalized ctx + rowsums
                            for hi in range(2):
                                stg = pbc.tile([1, fd_q], F32, tag="stg",
                                               name=f"rstg_{hp}_{qh}_{hi}")
                                nc.vector.tensor_copy(stg[:],
                                                      ctx[hi][DH:DH + 1, :])
                                nc.sync.dma_start(
                                    out=rs_p[hi * n_qh + qh:hi * n_qh + qh + 1, :],
                                    in_=stg[:])
                                nc.vector.tensor_copy(
                                    ctxT[hi * 64:(hi + 1) * 64,
                                         hp * s + qh * fd_q:
                                         hp * s + (qh + 1) * fd_q],
                                    ctx[hi][0:DH, :])

                        norm_pending = (hp, rs_p, rcp_p)
                    emit_norm(*norm_pending)

                # ---------------- phase C: out projection ----------------
                with (
                    tc.tile_pool(name="poolC", bufs=2) as pc,
                    tc.tile_pool(name="psC", bufs=4, space="PSUM") as psC,
                ):
                    for qc in range(n_sc):
                        ot = pc.tile([128, D], F32, tag="ot")
                        for n in range(2):
                            po = psC.tile([128, 512], F32, tag="po")
                            for ct in range(4):
                                nc.tensor.matmul(
                                    po[:],
                                    lhsT=ctxT[:, ct * s + qc * 128:
                                              ct * s + (qc + 1) * 128
                                              ],
                                    rhs=woutt[:, ct * D + n * 512:
                                              ct * D + (n + 1) * 512
                                              ],
                                    start=(ct == 0), stop=(ct == 3))
                            osl = ot[:, n * 512:(n + 1) * 512]
                            if n == 0:
                                nc.scalar.copy(out=osl, in_=po[:])
                            else:
                                nc.vector.tensor_copy(osl, po[:])
                        nc.sync.dma_start(
                            out=y[qc * 128:(qc + 1) * 128, :], in_=ot[:])

    _split_multiwait(nc)
    return nc


def _get_nc(s=S):
    if s not in _CACHE:
        _CACHE[s] = build_nc(s)
    return _CACHE[s]


def make_in_maps(x, W_qkv, b_qkv, W_out, mask, s=S):
    import ml_dtypes

    BF = ml_dtypes.bfloat16
    x = np.asarray(x, dtype=np.float32)
    W_qkv = np.asarray(W_qkv, dtype=np.float32)
    b_qkv = np.asarray(b_qkv, dtype=np.float32)
    W_out = np.asarray(W_out, dtype=np.float32)
    mask = np.asarray(mask)
    scale = 1.0 / np.sqrt(DH)
    m01 = np.ascontiguousarray((mask[0, 0] != 0).T.astype(BF))
    in_maps = []
    for c in range(NCORES):
        b, g = c // 2, c % 2
        wq = W_qkv[:, g * CD:(g + 1) * CD] * scale
        wk = W_qkv[:, D + g * CD:D + (g + 1) * CD]
        in_maps.append({
            "xT": np.ascontiguousarray(x[b].T.astype(BF)),
            "wqk": np.ascontiguousarray(
                np.concatenate([wq, wk], axis=1).astype(BF)),
            "wv": np.ascontiguousarray(
                W_qkv[:, 2 * D + g * CD:2 * D + (g + 1) * CD].astype(BF)),
            "bqk": np.ascontiguousarray(np.concatenate(
                [b_qkv[g * CD:(g + 1) * CD] * scale,
                 b_qkv[D + g * CD:D + (g + 1) * CD]])[None, :].astype(BF)),
            "bv": np.ascontiguousarray(
                b_qkv[2 * D + g * CD:2 * D + (g + 1) * CD][None, :].astype(BF)),
            "m01": m01,
            "wout": np.ascontiguousarray(
                W_out[g * CD:(g + 1) * CD, :].astype(BF)),
            "ones": np.ones((1, 512), dtype=BF),
        })
    return in_maps


def kernel(x, W_qkv, b_qkv, W_out, b_out, mask):
    from concourse.bass_utils import run_bass_kernel_spmd

    nc = _get_nc(S)
    in_maps = make_in_maps(x, W_qkv, b_qkv, W_out, mask, S)
    res = run_bass_kernel_spmd(nc, in_maps, list(range(NCORES)))
    b_out = np.asarray(b_out, dtype=np.float32)
    y = np.empty((B, S, D), dtype=np.float32)
    for b in range(B):
        y[b] = res.results[2 * b]["y"] + res.results[2 * b + 1]["y"] + b_out
    return y


# revision 47
# speedup vs baseline: 1.0735x; 1.0429x over previous
"""Multi-head attention (B=4, S=2048, D=1024, H=16) on 8 NeuronCores.

Sharding: core c -> (batch b = c//2, head-group g = c%2 of 8 heads).
Per-core: column-parallel fused qkv projection for its 8 heads,
flash-style attention (scores kept transposed: k on partitions so
softmax denominators come from a fused ones-column in the PV matmul),
row-parallel out-projection. The two partial outputs per batch are
summed on the host along with b_out.

Perf notes (876us -> 534us on trn2):
- all matmul operands bf16 (host-cast, halves input DMA); mask shipped
  as bf16 0/1 so the DVE mask-multiply hits its 2-byte fast path.
- attention inner loop (fd_q=512): both heads' scores land in one
  [128,1024] psum tile via quadrant-packed 64-contraction matmuls, so
  ONE exp on ACT and ONE rep-AP masked multiply on DVE cover the pair;
  ACT runs ~90% busy (exp is the attention floor).
- PV matmuls are deferred one k-chunk behind the scores so the
  in-order PE queue never blocks on the exp+mask round trip.
- scores psum triple-buffered (6 banks) + 2 ctx accumulators = 8.
- softmax normalization: rowsums from a fused ones-column in PV;
  reciprocal + bf16 row-broadcast via stride-0-partition DMA + DVE
  multiply, all deferred into the NEXT head-pair's attention (deep
  tile bufs to avoid WAR hazards on the deferred reads).
- vones ones-columns via memset (a DMA scatter here cost 50us);
  m01/x DMA split across both hwdge queues; wout prefetched at
  attention start; phase C writes y rows with single [128,1024] DMAs.
Failed experiments: Pool (gpsimd) for any elementwise op (3-4x slower
than DVE); co-resident quadrant pairs that write the same psum region
(hardware fault) or read different rhs streams (no gain - the win of
quadrant packing comes from sharing one rhs fetch stream); fp8 /
Schraudolph-approx exp (error budget); reciprocal_approx_fast (walrus
codegen "ISA wrong length").
"""
import sys

if "/opt/trn_rl_repo" not in sys.path:
    sys.path.insert(0, "/opt/trn_rl_repo")

import numpy as np

B, S, D, H = 4, 2048, 1024, 16
DH = D // H          # 64
HPC = H // 2         # 8 heads per core
CD = HPC * DH        # 512 local head-dims per core
NCORES = 8

_CACHE = {}


def _split_multiwait(nc):
    """walrus in this container accepts ONE sync wait per instruction;
    hoist extras onto injected same-engine EventSemaphore carriers."""
    import concourse.mybir as mybir

    for fn in nc.m.functions:
        for bb in fn.blocks:
            if not any(
                i.sync_info is not None and i.sync_info.on_wait
                and len(i.sync_info.on_wait) > 1
                for i in bb.instructions
            ):
                continue
            newlist = []
            for inst in bb.instructions:
                si = inst.sync_info
                if si is not None and si.on_wait and len(si.on_wait) > 1:
                    waits = list(si.on_wait)
                    for w in waits[:-1]:
                        ev = mybir.InstEventSemaphore(
                            name=nc.get_next_instruction_name(), ins=[], outs=[])
                        ev.engine = inst.engine
                        ev.sync_info = mybir.SyncInfo(on_wait=[w], on_update=[])
                        newlist.append(ev)
                    inst.sync_info = mybir.SyncInfo(
                        on_wait=[waits[-1]], on_update=list(si.on_update))
                newlist.append(inst)
            try:
                bb.instructions = newlist
            except Exception:
                bb.instructions.clear()
                bb.instructions.extend(newlist)


def build_nc(s=S):
    import concourse.bass as bass
    import concourse.mybir as mybir
    from concourse.tile import TileContext

    F32 = mybir.dt.float32
    F32R = mybir.dt.float32r
    BF16 = mybir.dt.bfloat16
    EXP = mybir.ActivationFunctionType.Exp
    MULT = mybir.AluOpType.mult

    n_sc = s // 128            # s-chunks of 128
    n_st = s // 512            # s-tiles of 512
    n_kc = s // 128            # k chunks (128 each)
    fd_q = min(512, s)         # q-tile width for attention inner loop
    n_qh = s // fd_q           # q tiles
    VW = CD + HPC              # vones row-chunk width (8 heads x 65)

    nc = bass.Bass("TRN2", num_devices=NCORES)

    xT = nc.declare_dram_parameter("xT", [D, s], BF16, isOutput=False)
    wqk = nc.declare_dram_parameter("wqk", [D, 2 * CD], BF16, isOutput=False)
    wv = nc.declare_dram_parameter("wv", [D, CD], BF16, isOutput=False)
    bqk = nc.declare_dram_parameter("bqk", [1, 2 * CD], BF16, isOutput=False)
    bv = nc.declare_dram_parameter("bv", [1, CD], BF16, isOutput=False)
    m01 = nc.declare_dram_parameter("m01", [s, s], BF16, isOutput=False)
    wout = nc.declare_dram_parameter("wout", [CD, D], BF16, isOutput=False)
    ones = nc.declare_dram_parameter("ones", [1, 512], BF16, isOutput=False)
    y = nc.declare_dram_parameter("y", [s, D], F32, isOutput=True)

    with TileContext(nc) as tc:
        with tc.tile_pool(name="persist", bufs=1) as pp:
            qkT = pp.tile([128, 8 * s], BF16, tag="qkT")       # [1024 c, s]
            vones = pp.tile([128, n_sc * VW], BF16, tag="vones")
            m01t = pp.tile([128, n_kc * s], BF16, tag="m01")

            # ---------------- phase A: qkv projection ----------------
            with tc.tile_pool(name="poolA", bufs=1) as pa, \
                 tc.tile_pool(name="psA", bufs=8, space="PSUM") as psA:
                xt = pa.tile([128, 8 * s], BF16, tag="xt")
                wqkt = pa.tile([128, 8 * 2 * CD], BF16, tag="wqkt")
                wvt = pa.tile([128, 8 * CD], BF16, tag="wvt")
                ones_row = pa.tile([1, 512], BF16, tag="ones")
                bqk_t = pa.tile([1, 2 * CD], BF16, tag="bqk")
                bv_t = pa.tile([1, CD], BF16, tag="bv")

                nc.sync.dma_start(out=ones_row[:], in_=ones[:])
                nc.sync.dma_start(out=bqk_t[:], in_=bqk[:])
                nc.sync.dma_start(out=bv_t[:], in_=bv[:])
                # x + qk weights first (feed the ct loop asap); x chunks
                # alternate queues so neither serializes the whole 4MB
                for dc in range(8):
                    nc.scalar.dma_start(
                        out=wqkt[:, dc * 2 * CD:(dc + 1) * 2 * CD],
                        in_=wqk[dc * 128:(dc + 1) * 128, :])
                    xeng = nc.sync if dc % 2 == 0 else nc.scalar
                    xeng.dma_start(out=xt[:, dc * s:(dc + 1) * s],
                                   in_=xT[dc * 128:(dc + 1) * 128, :])
                for dc in range(8):
                    nc.scalar.dma_start(out=wvt[:, dc * CD:(dc + 1) * CD],
                                        in_=wv[dc * 128:(dc + 1) * 128, :])
                # mask: needed only at attention start; queue behind x,
                # split across both hwdge queues
                for kc in range(n_kc):
                    eng = nc.sync if kc % 2 == 0 else nc.scalar
                    eng.dma_start(out=m01t[:, kc * s:(kc + 1) * s],
                                  in_=m01[kc * 128:(kc + 1) * 128, :])
                # ones columns of vones (the rest is overwritten below)
                vones_cols = vones[:].rearrange(
                    "p (ch e) -> p ch e", e=DH + 1)[:, :, DH:DH + 1]
                nc.gpsimd.memset(vones_cols, 1.0)

                # q/k: qkT[c, :] = (W.T x.T), c-tiles of 128
                for ct in range(8):
                    pst = [psA.tile([128, 512], F32, tag="pa",
                                    name=f"psqk_{ct}_{st}")
                           for st in range(n_st)]
                    for dc in range(8):
                        wsl = wqkt[:, dc * 2 * CD + ct * 128:
                                   dc * 2 * CD + (ct + 1) * 128]
                        for st in range(n_st):
                            nc.tensor.matmul(
                                pst[st][:],
                                lhsT=wsl,
                                rhs=xt[:, dc * s + st * 512:
                                       dc * s + (st + 1) * 512],
                                start=(dc == 0), stop=False)
                    for st in range(n_st):
                        nc.tensor.matmul(
                            pst[st][:],
                            lhsT=bqk_t[0:1, ct * 128:(ct + 1) * 128],
                            rhs=ones_row[0:1, :],
                            start=False, stop=True)
                        nc.scalar.copy(
                            out=qkT[:, ct * s + st * 512:ct * s + (st + 1) * 512],
                            in_=pst[st][:])

                # v: natural [s, c] layout, s-chunks of 128, fused ones col
                for scg in range(n_sc // 4):
                    psv = [psA.tile([128, 512], F32, tag="pa",
                                    name=f"psv_{scg}_{i}")
                           for i in range(4)]
                    for dc in range(8):
                        for sci in range(4):
                            sc = scg * 4 + sci
                            nc.tensor.matmul(
                                psv[sci][:],
                                lhsT=xt[:, dc * s + sc * 128:
                                        dc * s + (sc + 1) * 128],
                                rhs=wvt[:, dc * CD:(dc + 1) * CD],
                                start=(dc == 0), stop=False)
                    for sci in range(4):
                        sc = scg * 4 + sci
                        nc.tensor.matmul(
                            psv[sci][:],
                            lhsT=ones_row[0:1, 0:128],
                            rhs=bv_t[0:1, :],
                            start=False, stop=True)
                        dst = vones[:, sc * VW:(sc + 1) * VW].rearrange(
                            "p (h e) -> p h e", e=DH + 1)[:, :, 0:DH]
                        src = psv[sci][:].rearrange("p (h e) -> p h e", e=DH)
                        nc.vector.tensor_copy(dst, src)

            # ---------------- phase B: attention ----------------
            with tc.tile_pool(name="poolB", bufs=1) as pb:
                ctxT = pb.tile([128, 4 * s], BF16, tag="ctxT")   # [512 c, s]
                woutt = pb.tile([128, 4 * D], BF16, tag="wout")
                for ct in range(4):
                    nc.scalar.dma_start(out=woutt[:, ct * D:(ct + 1) * D],
                                        in_=wout[ct * 128:(ct + 1) * 128, :])
                with (
                    tc.tile_pool(name="poolE", bufs=16) as pe,
                    tc.tile_pool(name="poolBc", bufs=2) as pbc,
                    tc.tile_pool(name="psB_st", bufs=3, space="PSUM") as ps_st,
                    tc.tile_pool(name="psB_ctx", bufs=2, space="PSUM") as ps_ctx,
                ):
                    def emit_norm(hp, rs_p, rcp_p):
                        # normalize pair hp: ctxT[c, q] *= 1/rowsum,
                        # broadcasting the bf16 reciprocal rows over the 64
                        # partitions of each head with a stride-0 DMA
                        with nc.allow_low_precision(
                                reason="recip feeds bf16 prob scale"):
                            nc.vector.reciprocal(rcp_p[:], rs_p[:])
                        rcpb = pbc.tile([2 * n_qh, fd_q], BF16, tag="rcpb",
                                        bufs=4, name=f"rcpb_{hp}")
                        nc.vector.tensor_copy(rcpb[:], rcp_p[:])
                        for qh in range(n_qh):
                            bcp = pbc.tile([128, fd_q], BF16, tag="bcp",
                                           bufs=8, name=f"bcp_{hp}_{qh}")
                            for hi in range(2):
                                r = rcpb[hi * n_qh + qh:hi * n_qh + qh + 1, :]
                                rep = bass.AP(r.tensor, r.offset,
                                              [list(r.ap[0]), [0, 64],
                                               [1, fd_q]])
                                nc.sync.dma_start(
                                    out=bcp[hi * 64:(hi + 1) * 64, :], in_=rep)
                            sl = ctxT[:, hp * s + qh * fd_q:
                                      hp * s + (qh + 1) * fd_q]
                            nc.vector.tensor_tensor(sl, sl, bcp[:], MULT)

                    norm_pending = None
                    for hp in range(4):
                        h0, h1 = 2 * hp, 2 * hp + 1
                        kt_off = (4 + hp) * s   # K pair c-tile offset in qkT
                        qt_off = hp * s         # Q pair c-tile offset
                        rs_p = pbc.tile([2 * n_qh, fd_q], F32, tag="rsp",
                                        bufs=4, name=f"rs_{hp}")
                        rcp_p = pbc.tile([2 * n_qh, fd_q], F32, tag="rcpp",
                                         bufs=4, name=f"rcp_{hp}")
                        for qh in range(n_qh):
                            # previous pair's normalize drops in here, hidden
                            # under this pair's attention
                            if qh == 1 and norm_pending is not None:
                                emit_norm(*norm_pending)
                                norm_pending = None
                            ctx = [ps_ctx.tile([DH + 1, fd_q], F32, tag="ctx",
                                               name=f"ctx_{hp}_{qh}_{hi}")
                                   for hi in range(2)]

                            def emit_pv(kc, e2):
                                first, last = kc == 0, kc == n_kc - 1
                                for hi, h in enumerate((h0, h1)):
                                    nc.tensor.matmul(
                                        ctx[hi][:],
                                        lhsT=vones[:, kc * VW + h * (DH + 1):
                                                   kc * VW +
                                                   (h + 1) * (DH + 1)],
                                        rhs=e2[:, hi * 512:(hi + 1) * 512],
                                        start=first, stop=last)

                            # software pipeline: PV for k-chunk kc-1 issues
                            # after the scores for kc, so the in-order PE
                            # queue never blocks on the exp+mask round trip.
                            # Scores for both heads land in one psum tile
                            # (cols 0:512 head0 / 512:1024 head1) so one exp
                            # and one masked multiply cover the pair.
                            prev = None
                            for kc in range(n_kc):
                                pss = ps_st.tile([128, 2 * fd_q], F32,
                                                 tag="st")
                                for hi in range(2):
                                    r0, r1 = (0, 64) if hi == 0 else (64, 128)
                                    nc.tensor.matmul(
                                        pss[:, hi * 512:(hi + 1) * 512],
                                        lhsT=qkT[r0:r1,
                                                 kt_off + kc * 128:
                                                 kt_off + (kc + 1) * 128],
                                        rhs=qkT[r0:r1,
                                                qt_off + qh * fd_q:
                                                qt_off + (qh + 1) * fd_q],
                                        start=True, stop=True,
                                        tile_position=(r0, 0))
                                e = pe.tile([128, 2 * fd_q], BF16, tag="e")
                                nc.scalar.activation(e[:], pss[:], EXP)
                                mb = m01t[:, kc * s + qh * fd_q:
                                          kc * s + (qh + 1) * fd_q]
                                msl2 = bass.AP(mb.tensor, mb.offset,
                                               [list(mb.ap[0]), [0, 2],
                                                [1, fd_q]])
                                e2v = e[:].rearrange("p (a b) -> p a b", a=2)
                                nc.vector.tensor_tensor(e2v, e2v, msl2, MULT)
                                if prev is not None:
                                    emit_pv(kc - 1, prev)
                                prev = e
                            emit_pv(n_kc - 1, prev)
                            # spill unnormalized ctx + rowsums
                            for hi in range(2):
                                stg = pbc.tile([1, fd_q], F32, tag="stg",
                                               name=f"rstg_{hp}_{qh}_{hi}")
                                nc.vector.tensor_copy(stg[:],
                                                      ctx[hi][DH:DH + 1, :])
                                nc.sync.dma_start(
                                    out=rs_p[hi * n_qh + qh:hi * n_qh + qh + 1, :],
                                    in_=stg[:])
                                nc.vector.tensor_copy(
                                    ctxT[hi * 64:(hi + 1) * 64,
                                         hp * s + qh * fd_q:
                                         hp * s + (qh + 1) * fd_q],
                                    ctx[hi][0:DH, :])

                        norm_pending = (hp, rs_p, rcp_p)
                    emit_norm(*norm_pending)

                # ---------------- phase C: out projection ----------------
                with (
                    tc.tile_pool(name="poolC", bufs=2) as pc,
                    tc.tile_pool(name="psC", bufs=4, space="PSUM") as psC,
                ):
                    for qc in range(n_sc):
                        ot = pc.tile([128, D], F32, tag="ot")
                        for n in range(2):
                            po = psC.tile([128, 512], F32, tag="po")
                            for ct in range(4):
                                nc.tensor.matmul(
                                    po[:],
                                    lhsT=ctxT[:, ct * s + qc * 128:
                                              ct * s + (qc + 1) * 128
                                              ],
                                    rhs=woutt[:, ct * D + n * 512:
                                              ct * D + (n + 1) * 512
                                              ],
                                    start=(ct == 0), stop=(ct == 3))
                            osl = ot[:, n * 512:(n + 1) * 512]
                            if n == 0:
                                nc.scalar.copy(out=osl, in_=po[:])
                            else:
                                nc.vector.tensor_copy(osl, po[:])
                        nc.sync.dma_start(
                            out=y[qc * 128:(qc + 1) * 128, :], in_=ot[:])

    _split_multiwait(nc)
    return nc


def _get_nc(s=S):
    if s not in _CACHE:
        _CACHE[s] = build_nc(s)
    return _CACHE[s]


def make_in_maps(x, W_qkv, b_qkv, W_out, mask, s=S):
    import ml_dtypes

    BF = ml_dtypes.bfloat16
    x = np.asarray(x, dtype=np.float32)
    W_qkv = np.asarray(W_qkv, dtype=np.float32)
    b_qkv = np.asarray(b_qkv, dtype=np.float32)
    W_out = np.asarray(W_out, dtype=np.float32)
    mask = np.asarray(mask)
    scale = 1.0 / np.sqrt(DH)
    m01 = np.ascontiguousarray((mask[0, 0] != 0).T.astype(BF))
    in_maps = []
    for c in range(NCORES):
        b, g = c // 2, c % 2
        wq = W_qkv[:, g * CD:(g + 1) * CD] * scale
        wk = W_qkv[:, D + g * CD:D + (g + 1) * CD]
        in_maps.append({
            "xT": np.ascontiguousarray(x[b].T.astype(BF)),
            "wqk": np.ascontiguousarray(
                np.concatenate([wq, wk], axis=1).astype(BF)),
            "wv": np.ascontiguousarray(
                W_qkv[:, 2 * D + g * CD:2 * D + (g + 1) * CD].astype(BF)),
            "bqk": np.ascontiguousarray(np.concatenate(
                [b_qkv[g * CD:(g + 1) * CD] * scale,
                 b_qkv[D + g * CD:D + (g + 1) * CD]])[None, :].astype(BF)),
            "bv": np.ascontiguousarray(
                b_qkv[2 * D + g * CD:2 * D + (g + 1) * CD][None, :].astype(BF)),
            "m01": m01,
            "wout": np.ascontiguousarray(
                W_out[g * CD:(g + 1) * CD, :].astype(BF)),
            "ones": np.ones((1, 512), dtype=BF),
        })
    return in_maps


def kernel(x, W_qkv, b_qkv, W_out, b_out, mask):
    from concourse.bass_utils import run_bass_kernel_spmd

    nc = _get_nc(S)
    in_maps = make_in_maps(x, W_qkv, b_qkv, W_out, mask, S)
    res = run_bass_kernel_spmd(nc, in_maps, list(range(NCORES)))
    b_out = np.asarray(b_out, dtype=np.float32)
    y = np.empty((B, S, D), dtype=np.float32)
    for b in range(B):
        y[b] = res.results[2 * b]["y"] + res.results[2 * b + 1]["y"] + b_out
    return y
